# revision 1
# baseline (speedup 1.0000x reference)
"""AttentativeResidual Trainium2 kernel.

out[b,t,n,:] = x[b,t,n,:] + softmax_m(x[b,t,n,:] @ Wq @ Wk^T @ rs[b]^T) @ (rs[b] @ Wv)

Shapes: x [4,8,2048,128], residual_source [4,2048,128], W* [128,128], fp32.

Sharding: data-parallel over (b,t): core i handles b = i//2, t in
[(i%2)*4, (i%2)*4+4). Each core sees one batch b, so the per-batch
K/V-side work (rs^T, G = A @ rs^T, v = rs @ Wv) is computed once per core.

Math trick: fold A = Wq @ Wk^T (host-side, [128,128]) so the logits are
x @ A @ rs^T; on-device GT = A @ rs^T ([c,m]) replaces both q and k
projections.

Device algorithm per core (flash-attention style, transposed logits):
  setup:  rsT (PE transposes), GT = A@rsT (fp32 matmul, cast fp16),
          v_aug = [rs@Wv | 1] (fp32 matmul, cast bf16, ones col)
  per t:  xT via PE transposes (cast fp16)
          per m-tile (16): affT[m-part, r] = GT_m^T @ xT  (fp16 matmul,
             fp32 psum [128,1024] x2) ; ea[m] = exp(affT) -> bf16 sbuf
          per r-subtile (16): av[r,129] = sum_m ea[m][:,rsub]^T @ v_aug[m]
             (bf16 matmul, accumulated in psum; col 128 = softmax denom)
          out[r,:] = av[:, :128] * (1/av[:,128]) + x[r,:]

exp is computed without max-subtraction: logits ~ N(0, 128), |l| < ~75
with overwhelming probability, exp fits fp32/bf16 range. ea/v in bf16
because unnormalized exp(l) overflows fp16.
"""
import numpy as np

import concourse.bacc as bacc
import concourse.tile as tile
import concourse.mybir as mybir
from concourse.bass_utils import run_bass_kernel_spmd
from concourse.masks import make_identity

F32 = mybir.dt.float32
F16 = mybir.dt.float16
F32R = mybir.dt.float32r
BF16 = mybir.dt.bfloat16
EXP = mybir.ActivationFunctionType.Exp

B, T, N, C = 4, 8, 2048, 128
NCORES = 8
TPC = (B * T) // NCORES          # (b,t) pairs per core = 4
NT = N // 128                    # 16 row/key tiles


def _body(ctx, tc, xs, rs, at, wv, out):
    nc = tc.nc
    const = ctx.enter_context(tc.tile_pool(name="const", bufs=1))
    xpool = ctx.enter_context(tc.tile_pool(name="xp", bufs=2))
    xtp = ctx.enter_context(tc.tile_pool(name="xtp", bufs=2))
    eap = ctx.enter_context(tc.tile_pool(name="eap", bufs=34))
    outp = ctx.enter_context(tc.tile_pool(name="outp", bufs=3))
    recp = ctx.enter_context(tc.tile_pool(name="recp", bufs=4))
    psA = ctx.enter_context(tc.tile_pool(name="psA", bufs=2, space="PSUM"))
    psB = ctx.enter_context(tc.tile_pool(name="psB", bufs=2, space="PSUM"))
    psC = ctx.enter_context(tc.tile_pool(name="psC", bufs=2, space="PSUM"))

    xr = xs[:, :, :].rearrange("t (i p) c -> t p i c", p=128)
    outr = out[:, :, :].rearrange("t (i p) c -> t p i c", p=128)

    ident = const.tile([128, 128], F32, tag="ident")
    make_identity(nc, ident)
    at_sb = const.tile([128, 128], F32, tag="at")
    wv_sb = const.tile([128, 128], F32, tag="wv")
    nc.sync.dma_start(out=at_sb, in_=at[:, :])
    nc.sync.dma_start(out=wv_sb, in_=wv[:, :])
    at_r = const.tile([128, 128], F32R, tag="atr")
    wv_r = const.tile([128, 128], F32R, tag="wvr")
    nc.vector.tensor_copy(out=at_r, in_=at_sb)
    nc.vector.tensor_copy(out=wv_r, in_=wv_sb)

    # --- per-batch setup: rsT, GT (fp16), v_aug (bf16, ones col) ---
    rs_sb = const.tile([128, NT, 128], F32, tag="rs")
    nc.sync.dma_start(out=rs_sb, in_=rs[:, :].rearrange("(i p) c -> p i c", p=128))
    rsT_sb = const.tile([128, NT, 128], F32R, tag="rsT")
    for i in range(NT):
        tp = psC.tile([128, 512], F32, tag="misc")
        nc.tensor.transpose(tp[:, 0:128], rs_sb[:, i, :], ident)
        nc.vector.tensor_copy(out=rsT_sb[:, i, :], in_=tp[:, 0:128])

    gt_sb = const.tile([128, NT, 128], F32R, tag="gt")
    for j in range(4):
        gp = psC.tile([128, 512], F32, tag="misc")
        nc.tensor.matmul(gp, at_r, rsT_sb[:, 4 * j:4 * j + 4, :],
                         start=True, stop=True)
        nc.vector.tensor_copy(out=gt_sb[:, 4 * j:4 * j + 4, :], in_=gp)

    v_aug = const.tile([128, NT, 129], BF16, tag="vaug")

    def emit_v():
        for i in range(NT):
            vp = psC.tile([128, 512], F32, tag="misc")
            nc.tensor.matmul(vp[:, 0:128], rsT_sb[:, i, :], wv_r,
                             start=True, stop=True)
            nc.vector.tensor_copy(out=v_aug[:, i, 0:128], in_=vp[:, 0:128])
        nc.vector.memset(v_aug[:, :, 128:129], 1.0)

    # --- main loop over the 4 (b,t) pairs, software-pipelined at r-half
    # granularity: PE runs the AV phase of the previous half while ACT is
    # still exp-ing the current one, keeping both engines saturated.
    def emit_av(ph):
        t, rb, eas_h, x_sb_h = ph
        out_sb = outp.tile([128, 8, 128], F32, tag="o")
        for rs8 in range(8):
            rsub = 8 * rb + rs8
            av = psB.tile([128, 129], F32, tag="av")
            for m in range(NT):
                nc.tensor.matmul(av, eas_h[m][:, 128 * rs8:128 * (rs8 + 1)],
                                 v_aug[:, m, :],
                                 start=(m == 0), stop=(m == NT - 1))
            rec = recp.tile([128, 1], F32, tag="rec")
            nc.vector.reciprocal(out=rec, in_=av[:, 128:129])
            nc.vector.tensor_scalar_mul(out=out_sb[:, rs8, :],
                                        in0=av[:, 0:128], scalar1=rec)
            nc.vector.tensor_add(out=out_sb[:, rs8, :],
                                 in0=out_sb[:, rs8, :],
                                 in1=x_sb_h[:, rsub, :])
        nc.sync.dma_start(out=outr[t][:, 8 * rb:8 * (rb + 1), :], in_=out_sb)

    prev = None
    for t in range(TPC):
        x_sb = xpool.tile([128, NT, 128], F32, tag="x")
        nc.sync.dma_start(out=x_sb, in_=xr[t])

        xt_sb = xtp.tile([128, NT, 128], F32R, tag="xt")
        for i in range(NT):
            tp = psC.tile([128, 512], F32, tag="misc")
            nc.tensor.transpose(tp[:, 0:128], x_sb[:, i, :], ident)
            nc.vector.tensor_copy(out=xt_sb[:, i, :], in_=tp[:, 0:128])

        for rb in range(2):
            eas_h = []
            for m in range(NT):
                ea = eap.tile([128, 1024], BF16, tag="ea")
                ap = psA.tile([128, 1024], F32, tag="aff")
                for jj in range(2):
                    nc.tensor.matmul(
                        ap[:, 512 * jj:512 * (jj + 1)],
                        gt_sb[:, m, :],
                        xt_sb[:, 8 * rb + 4 * jj:8 * rb + 4 * (jj + 1), :],
                        start=True, stop=True)
                nc.scalar.activation(out=ea, in_=ap, func=EXP)
                eas_h.append(ea)
            if prev is None:
                emit_v()
            else:
                emit_av(prev)
            prev = (t, rb, eas_h, x_sb)
    emit_av(prev)


def _run_on_cores(nc, in_maps):
    """Run the bass module on len(in_maps) NeuronCores as independent
    single-device programs dispatched concurrently.

    run_bass_kernel_spmd's multi-core path lowers to one shard_map program
    spanning 8 devices, which deadlocks through the axon PJRT tunnel in this
    environment. Independent per-device jits of the same bass_exec body work
    (and still run concurrently on all 8 cores), so we dispatch those.
    """
    import jax
    from concourse import bass2jax

    bass2jax.install_neuronx_cc_hook()
    devices = jax.devices()[:len(in_maps)]
    assert len(devices) == len(in_maps)

    partition_name = (nc.partition_id_tensor.name
                      if nc.partition_id_tensor else None)
    dbg_name = nc.dbg_addr.name if nc.dbg_addr is not None else None
    in_names, out_names, out_avals, zero_outs = [], [], [], []
    for alloc in nc.m.functions[0].allocations:
        if not isinstance(alloc, mybir.MemoryLocationSet):
            continue
        name = alloc.memorylocations[0].name
        if alloc.kind == "ExternalInput":
            if name != partition_name:
                in_names.append(name)
        elif alloc.kind == "ExternalOutput":
            shape = tuple(alloc.tensor_shape)
            dtype = mybir.dt.np(alloc.dtype)
            out_names.append(name)
            out_avals.append(jax.core.ShapedArray(shape, dtype))
            zero_outs.append(np.zeros(shape, dtype))

    n_params = len(in_names)
    in_names_all = tuple(in_names + out_names + (
        [partition_name] if partition_name else []))
    donate = tuple(range(n_params, n_params + len(out_names)))

    def _bass_body(*args):
        operands = list(args)
        if partition_name is not None:
            operands.append(bass2jax.partition_id_tensor())
        outs = bass2jax._bass_exec_p.bind(
            *operands,
            out_avals=tuple(out_avals),
            in_names=in_names_all,
            out_names=tuple(out_names),
            lowering_input_output_aliases=(),
            sim_require_finite=True,
            sim_require_nnan=True,
            nc=nc,
        )
        return tuple(outs)

    jf = jax.jit(_bass_body, donate_argnums=donate, keep_unused=True)
    futs = []
    for c, im in enumerate(in_maps):
        im = dict(im)
        if dbg_name is not None:
            im[dbg_name] = np.zeros((1, 2), np.uint32)
        args = [jax.device_put(np.asarray(im[n]), devices[c])
                for n in in_names]
        args += [jax.device_put(z, devices[c]) for z in zero_outs]
        futs.append(jf(*args))
    return [{n: np.asarray(outs[i]) for i, n in enumerate(out_names)}
            for outs in futs]


_NC_CACHE = None


def _get_nc():
    global _NC_CACHE
    if _NC_CACHE is None:
        nc = bacc.Bacc("TRN2", target_bir_lowering=False)
        xs = nc.dram_tensor("xs", [TPC, N, C], F32, kind="ExternalInput")
        rs = nc.dram_tensor("rs", [N, C], F32, kind="ExternalInput")
        at = nc.dram_tensor("at", [C, C], F32, kind="ExternalInput")
        wv = nc.dram_tensor("wv", [C, C], F32, kind="ExternalInput")
        out = nc.dram_tensor("out", [TPC, N, C], F32, kind="ExternalOutput")
        from contextlib import ExitStack
        with tile.TileContext(nc) as tc, ExitStack() as ctx:
            _body(ctx, tc, xs, rs, at, wv, out)
        nc.finalize()
        _NC_CACHE = nc
    return _NC_CACHE


def kernel(x, residual_source, Wq, Wk, Wv):
    x = np.asarray(x, dtype=np.float32)
    residual_source = np.asarray(residual_source, dtype=np.float32)
    Wq = np.asarray(Wq, dtype=np.float32)
    Wk = np.asarray(Wk, dtype=np.float32)
    Wv = np.asarray(Wv, dtype=np.float32)

    at = np.ascontiguousarray(Wk @ Wq.T)  # A^T where A = Wq @ Wk^T
    nc = _get_nc()

    in_maps = []
    for core in range(NCORES):
        b, toff = core // 2, (core % 2) * TPC
        in_maps.append({
            "xs": np.ascontiguousarray(x[b, toff:toff + TPC]),
            "rs": np.ascontiguousarray(residual_source[b]),
            "at": at,
            "wv": np.ascontiguousarray(Wv),
        })
    results = _run_on_cores(nc, in_maps)

    out = np.empty((B, T, N, C), np.float32)
    for core in range(NCORES):
        b, toff = core // 2, (core % 2) * TPC
        out[b, toff:toff + TPC] = results[core]["out"]
    return out


if __name__ == "__main__":
    rng = np.random.default_rng(0)
    x = rng.standard_normal((B, T, N, C)).astype(np.float32)
    rs = rng.standard_normal((B, N, C)).astype(np.float32)
    s = 1.0 / np.sqrt(C)
    Wq = (rng.standard_normal((C, C)) * s).astype(np.float32)
    Wk = (rng.standard_normal((C, C)) * s).astype(np.float32)
    Wv = (rng.standard_normal((C, C)) * s).astype(np.float32)
    y = kernel(x, rs, Wq, Wk, Wv)
    print("out", y.shape, y.dtype)



# revision 10
# speedup vs baseline: 1.1736x; 1.1736x over previous
"""AttentativeResidual Trainium2 kernel.

out[b,t,n,:] = x[b,t,n,:] + softmax_m(x[b,t,n,:] @ Wq @ Wk^T @ rs[b]^T) @ (rs[b] @ Wv)

Shapes: x [4,8,2048,128], residual_source [4,2048,128], W* [128,128], fp32.

Sharding: data-parallel over (b,t): core i handles b = i//2, t in
[(i%2)*4, (i%2)*4+4). Each core sees one batch b, so the per-batch
K/V-side work (rs^T, G = A @ rs^T, v = rs @ Wv) is computed once per core.

Math trick: fold A = Wq @ Wk^T (host-side, [128,128]) so the logits are
x @ A @ rs^T; on-device GT = A @ rsT ([c,m]) replaces both q and k
projections.

Device algorithm per core (flash-attention style, transposed logits):
  setup:  rsT (PE transposes, fp32r), GT = A@rsT (fp32r), v_aug =
          [rs@Wv | 1] (bf16 matmul, ones col)
  per (t, rb-half): per m-tile (16): aff^T[m-part, n] = GT_m^T @ xT
          (fp32r matmul, fp32 psum [128,1024]); exp -> bf16 ea tile,
          split across engines: ACT does true exp on 12/16 tiles; DVE
          and Pool each do 2/16 via a Schraudolph bit-trick (see below)
  AV (software-pipelined into the NEXT half's m-slots so PE never
  idles): per r-subtile: av[r,129] = sum_m ea[m][:,rsub]^T @ v_aug[m]
          (bf16, psum-accumulated; col 128 = softmax denominator)
          out[r,:] = av[:,:128] * (1/av[:,128]) + x[r,:]  (one fused
          DVE pass via the stock AFFINE_THEN_ADD custom op)

exp is computed without max-subtraction: logits ~ N(0, 128), |l| < ~75
with overwhelming probability, exp fits fp32/bf16 range. ea/v in bf16
because unnormalized exp(l) overflows fp16.

Fast exp (DVE/Pool tiles): exp(l) ~= bitcast_bf16(int16(l * 128*log2e
+ 127*128 + c)). The fp32->int16 output conversion truncates, so the
+0.5 is folded into c; the int16 bit pattern IS the bf16 exponent+
mantissa of 2^(l*log2e) with the mantissa linearly interpolated
(~+-4% sawtooth). End-to-end rel err measured 0.9e-2 vs the 2e-2 gate
(softmax weights are ratios, so the common-mode part cancels).

Schedule per (t, rb): the m-loop interleaves, per slot: 1 aff matmul
(1024 cols), the exp for that tile, and on odd slots one AV r-chunk of
the PREVIOUS half (16 accumulating matmuls); rb==1 slots also carry one
transpose of the next t's x tile. This keeps PE (the bottleneck at
~14.3us/half) fed while psA double-buffering paces the exp engines.
"""
import numpy as np

import concourse.bacc as bacc
import concourse.tile as tile
import concourse.mybir as mybir
from concourse.bass_utils import run_bass_kernel_spmd
from concourse.dve_ops import AFFINE_THEN_ADD
from concourse.masks import make_identity

F32 = mybir.dt.float32
F32R = mybir.dt.float32r
I16 = mybir.dt.int16
BF16 = mybir.dt.bfloat16
EXP = mybir.ActivationFunctionType.Exp
MULT = mybir.AluOpType.mult
ADD = mybir.AluOpType.add

B, T, N, C = 4, 8, 2048, 128
NCORES = 8
TPC = (B * T) // NCORES          # (b,t) pairs per core = 4
NT = N // 128                    # 16 row/key tiles

# Schraudolph fast-exp constants (bf16 bit pattern via int16 convert).
EXP_A = float(128.0 * np.log2(np.e))
EXP_B = float(127.0 * 128.0 + 0.5 - 6.0)
# m-slot -> exp engine: 'A' = ACT true exp, 'D' = DVE fast, 'P' = Pool fast.
EXP_ENG = ['A'] * NT
EXP_ENG[3] = EXP_ENG[6] = EXP_ENG[9] = EXP_ENG[12] = EXP_ENG[15] = 'D'


def _body(ctx, tc, xs, rs, at, wv, out):
    nc = tc.nc
    const = ctx.enter_context(tc.tile_pool(name="const", bufs=1))
    xpool = ctx.enter_context(tc.tile_pool(name="xp", bufs=2))
    xtp = ctx.enter_context(tc.tile_pool(name="xtp", bufs=2))
    eap = ctx.enter_context(tc.tile_pool(name="eap", bufs=34))
    outp = ctx.enter_context(tc.tile_pool(name="outp", bufs=3))
    recp = ctx.enter_context(tc.tile_pool(name="recp", bufs=8))
    # psA: 4 x 1-bank aff tiles -- the 4-deep rotation halves the
    # aff->exp->aff WAR chain latency vs 2 x [128,1024].
    psA = ctx.enter_context(tc.tile_pool(name="psA", bufs=4, space="PSUM"))
    psB = ctx.enter_context(tc.tile_pool(name="psB", bufs=2, space="PSUM"))
    psC = ctx.enter_context(tc.tile_pool(name="psC", bufs=2, space="PSUM"))

    xr = xs[:, :, :].rearrange("t (i p) c -> t p i c", p=128)
    outr = out[:, :, :].rearrange("t (i p) c -> t p i c", p=128)

    ident = const.tile([128, 128], F32, tag="ident")
    make_identity(nc, ident)
    # at/wv go first on the ACT HWDGE queue (tiny, and their DVE copies
    # head the DVE queue); rs follows in 4 chunks so the first rsT
    # transposes start ~2us in; x goes on the SP queue in parallel.
    at_sb = const.tile([128, 128], F32, tag="at")
    wv_sb = const.tile([128, 128], F32, tag="wv")
    nc.scalar.dma_start(out=at_sb, in_=at[:, :])
    nc.scalar.dma_start(out=wv_sb, in_=wv[:, :])
    rs_sb = const.tile([128, NT, 128], F32, tag="rs")
    rsr = rs[:, :].rearrange("(i p) c -> p i c", p=128)
    for j in range(4):
        nc.scalar.dma_start(out=rs_sb[:, 4 * j:4 * j + 4, :],
                            in_=rsr[:, 4 * j:4 * j + 4, :])
    at_r = const.tile([128, 128], F32R, tag="atr")
    wv_r = const.tile([128, 128], F32R, tag="wvr")
    nc.vector.tensor_copy(out=at_r, in_=at_sb)
    nc.vector.tensor_copy(out=wv_r, in_=wv_sb)

    xss = {}
    xts = {}

    def issue_x_dma(t, split=1):
        xss[t] = xpool.tile([128, NT, 128], F32, tag="x", name="x_sb")
        step = NT // split
        for j in range(split):
            nc.sync.dma_start(out=xss[t][:, step * j:step * (j + 1), :],
                              in_=xr[t][:, step * j:step * (j + 1), :])

    tgroups = {}

    def emit_transpose(t, i, pool, tag, copy_eng):
        # Groups of 4 transposes share one psum tile; a single 512-col
        # copy drains the group (fewer, wider DVE instructions).
        g = tgroups.get((pool is psA, tag))
        if g is None or g[2] == 4:
            tp = pool.tile([128, 512], F32, tag=tag, name="tp")
            g = [tp, i, 0]
        nc.tensor.transpose(g[0][:, 128 * g[2]:128 * (g[2] + 1)],
                            xss[t][:, i, :], ident)
        g[2] += 1
        tgroups[(pool is psA, tag)] = g
        if g[2] == 4:
            i0 = g[1]
            copy_eng.tensor_copy(out=xts[t][:, i0:i0 + 4, :], in_=g[0])

    issue_x_dma(0, split=4)

    # --- per-batch setup: rsT (fp32r), GT (fp32r); x0 transposes
    # interleave on the psA pool so both PSUM chains run in parallel.
    # GT(j) is emitted as soon as its 4 rsT tiles land; GT psum->SBUF
    # copies go to ACT (idle during setup) to keep DVE off the chain.
    rsT_sb = const.tile([128, NT, 128], F32R, tag="rsT")
    gt_sb = const.tile([128, NT, 128], F32R, tag="gt")
    xts[0] = xtp.tile([128, NT, 128], F32R, tag="xt", name="xt_sb")
    rtp = None
    for i in range(NT):
        if i % 4 == 0:
            rtp = psC.tile([128, 512], F32, tag="misc", name="rtp")
        nc.tensor.transpose(rtp[:, 128 * (i % 4):128 * (i % 4 + 1)],
                            rs_sb[:, i, :], ident)
        if i < 8:
            emit_transpose(0, i, psA, "aff", nc.vector)
        if i % 4 == 3:
            j = i // 4
            nc.vector.tensor_copy(out=rsT_sb[:, 4 * j:4 * j + 4, :], in_=rtp)
            gp = psC.tile([128, 512], F32, tag="misc")
            nc.tensor.matmul(gp, at_r, rsT_sb[:, 4 * j:4 * j + 4, :],
                             start=True, stop=True)
            nc.vector.tensor_copy(out=gt_sb[:, 4 * j:4 * j + 4, :], in_=gp)

    # v = rs @ Wv runs in fp32r during the ACT-bound first half (PE has
    # slack there); v_aug holds [v | 1] with the softmax-denominator col.
    v_aug = const.tile([128, NT, 129], BF16, tag="vaug")

    vgroup = [None]

    def emit_v(i0, i1):
        for i in range(i0, i1):
            if i % 4 == 0:
                vgroup[0] = psC.tile([128, 512], F32, tag="misc", name="vp")
            vp = vgroup[0]
            nc.tensor.matmul(vp[:, 128 * (i % 4):128 * (i % 4 + 1)],
                             rsT_sb[:, i, :], wv_r, start=True, stop=True)
            if i % 4 == 3:
                nc.vector.tensor_copy(out=v_aug[:, i - 3:i + 1, 0:128], in_=vp)
        if i1 == NT:
            nc.vector.memset(v_aug[:, :, 128:129], 1.0)

    # One AV r-chunk of a previous half: 16 accumulating bf16 matmuls,
    # then out = av * (1/denominator) + x in one fused DVE op. Emitted in
    # two pieces (two consecutive m-slots) so PE work stays evenly paced.
    av_live = {}

    def emit_av_piece(ph, k, piece):
        t_, rb_, eas, x_sb_, out_sb = ph
        if piece == 0:
            av_live[k] = psB.tile([128, 129], F32, tag="av", name="av")
        av = av_live[k]
        m0 = piece * (NT // 2)
        for m in range(m0, m0 + NT // 2):
            nc.tensor.matmul(av, eas[m][:, 128 * k:128 * (k + 1)],
                             v_aug[:, m, :],
                             start=(m == 0), stop=(m == NT - 1))
        if piece == 1:
            rec = recp.tile([128, 1], F32, tag="rec")
            nc.vector.reciprocal(out=rec, in_=av[:, 128:129])
            nc.vector._custom_dve(AFFINE_THEN_ADD, out=out_sb[:, k, :],
                                  in0=av[:, 0:128],
                                  in1=x_sb_[:, 8 * rb_ + k, :],
                                  s0=rec, s1=0.0)
            if k == 7:
                nc.sync.dma_start(out=outr[t_][:, 8 * rb_:8 * (rb_ + 1), :],
                                  in_=out_sb)

    def emit_exp(eng, ea_half, ap):
        if eng == 'A':
            nc.scalar.activation(out=ea_half, in_=ap, func=EXP)
        else:
            veng = nc.vector if eng == 'D' else nc.gpsimd
            veng.tensor_scalar(out=ea_half.bitcast(I16), in0=ap,
                               scalar1=EXP_A, scalar2=EXP_B,
                               op0=MULT, op1=ADD)

    prev = None
    for t in range(TPC):
        if t + 1 < TPC:
            issue_x_dma(t + 1)
        for rb in range(2):
            if rb == 1 and t + 1 < TPC:
                xts[t + 1] = xtp.tile([128, NT, 128], F32R, tag="xt",
                                      name="xt_sb")
            eas_h = []
            for m in range(NT):
                ea = eap.tile([128, 1024], BF16, tag="ea")
                eng = EXP_ENG[m]
                for h in range(2):
                    ap = psA.tile([128, 512], F32, tag="aff", name="ap")
                    nc.tensor.matmul(
                        ap, gt_sb[:, m, :],
                        xts[t][:, 8 * rb + 4 * h:8 * rb + 4 * (h + 1), :],
                        start=True, stop=True)
                    emit_exp(eng, ea[:, 512 * h:512 * (h + 1)], ap)
                eas_h.append(ea)
                if prev is not None:
                    emit_av_piece(prev, m // 2, m % 2)
                else:
                    if m % 2 == 1:
                        emit_v(m - 1, m + 1)
                    if m % 2 == 0:
                        emit_transpose(0, 8 + m // 2, psC, "misc", nc.vector)
                if rb == 1 and t + 1 < TPC:
                    emit_transpose(t + 1, m, psC, "misc", nc.vector)
            out_sb = outp.tile([128, 8, 128], F32, tag="o")
            prev = (t, rb, eas_h, xss[t], out_sb)
    for k in range(8):
        emit_av_piece(prev, k, 0)
        emit_av_piece(prev, k, 1)


def _run_on_cores(nc, in_maps):
    """Run the bass module on len(in_maps) NeuronCores as independent
    single-device programs dispatched concurrently.

    run_bass_kernel_spmd's multi-core path lowers to one shard_map program
    spanning 8 devices, which deadlocks through the axon PJRT tunnel in this
    environment. Independent per-device jits of the same bass_exec body work
    (and still run concurrently on all 8 cores), so we dispatch those.
    """
    import jax
    from concourse import bass2jax

    bass2jax.install_neuronx_cc_hook()
    devices = jax.devices()[:len(in_maps)]
    assert len(devices) == len(in_maps)

    partition_name = (nc.partition_id_tensor.name
                      if nc.partition_id_tensor else None)
    dbg_name = nc.dbg_addr.name if nc.dbg_addr is not None else None
    in_names, out_names, out_avals, zero_outs = [], [], [], []
    for alloc in nc.m.functions[0].allocations:
        if not isinstance(alloc, mybir.MemoryLocationSet):
            continue
        name = alloc.memorylocations[0].name
        if alloc.kind == "ExternalInput":
            if name != partition_name:
                in_names.append(name)
        elif alloc.kind == "ExternalOutput":
            shape = tuple(alloc.tensor_shape)
            dtype = mybir.dt.np(alloc.dtype)
            out_names.append(name)
            out_avals.append(jax.core.ShapedArray(shape, dtype))
            zero_outs.append(np.zeros(shape, dtype))

    n_params = len(in_names)
    in_names_all = tuple(in_names + out_names + (
        [partition_name] if partition_name else []))
    donate = tuple(range(n_params, n_params + len(out_names)))

    def _bass_body(*args):
        operands = list(args)
        if partition_name is not None:
            operands.append(bass2jax.partition_id_tensor())
        outs = bass2jax._bass_exec_p.bind(
            *operands,
            out_avals=tuple(out_avals),
            in_names=in_names_all,
            out_names=tuple(out_names),
            lowering_input_output_aliases=(),
            sim_require_finite=True,
            sim_require_nnan=True,
            nc=nc,
        )
        return tuple(outs)

    jf = jax.jit(_bass_body, donate_argnums=donate, keep_unused=True)
    futs = []
    for c, im in enumerate(in_maps):
        im = dict(im)
        if dbg_name is not None:
            im[dbg_name] = np.zeros((1, 2), np.uint32)
        args = [jax.device_put(np.asarray(im[n]), devices[c])
                for n in in_names]
        args += [jax.device_put(z, devices[c]) for z in zero_outs]
        futs.append(jf(*args))
    return [{n: np.asarray(outs[i]) for i, n in enumerate(out_names)}
            for outs in futs]


_NC_CACHE = None


def _get_nc():
    global _NC_CACHE
    if _NC_CACHE is None:
        nc = bacc.Bacc("TRN2", target_bir_lowering=False)
        xs = nc.dram_tensor("xs", [TPC, N, C], F32, kind="ExternalInput")
        rs = nc.dram_tensor("rs", [N, C], F32, kind="ExternalInput")
        at = nc.dram_tensor("at", [C, C], F32, kind="ExternalInput")
        wv = nc.dram_tensor("wv", [C, C], F32, kind="ExternalInput")
        out = nc.dram_tensor("out", [TPC, N, C], F32, kind="ExternalOutput")
        from contextlib import ExitStack
        with tile.TileContext(nc) as tc, ExitStack() as ctx:
            _body(ctx, tc, xs, rs, at, wv, out)
        nc.finalize()
        _NC_CACHE = nc
    return _NC_CACHE


def kernel(x, residual_source, Wq, Wk, Wv):
    x = np.asarray(x, dtype=np.float32)
    residual_source = np.asarray(residual_source, dtype=np.float32)
    Wq = np.asarray(Wq, dtype=np.float32)
    Wk = np.asarray(Wk, dtype=np.float32)
    Wv = np.asarray(Wv, dtype=np.float32)

    at = np.ascontiguousarray(Wk @ Wq.T)  # A^T where A = Wq @ Wk^T
    nc = _get_nc()

    in_maps = []
    for core in range(NCORES):
        b, toff = core // 2, (core % 2) * TPC
        in_maps.append({
            "xs": np.ascontiguousarray(x[b, toff:toff + TPC]),
            "rs": np.ascontiguousarray(residual_source[b]),
            "at": at,
            "wv": np.ascontiguousarray(Wv),
        })
    results = _run_on_cores(nc, in_maps)

    out = np.empty((B, T, N, C), np.float32)
    for core in range(NCORES):
        b, toff = core // 2, (core % 2) * TPC
        out[b, toff:toff + TPC] = results[core]["out"]
    return out


if __name__ == "__main__":
    rng = np.random.default_rng(0)
    x = rng.standard_normal((B, T, N, C)).astype(np.float32)
    rs = rng.standard_normal((B, N, C)).astype(np.float32)
    s = 1.0 / np.sqrt(C)
    Wq = (rng.standard_normal((C, C)) * s).astype(np.float32)
    Wk = (rng.standard_normal((C, C)) * s).astype(np.float32)
    Wv = (rng.standard_normal((C, C)) * s).astype(np.float32)
    y = kernel(x, rs, Wq, Wk, Wv)
    print("out", y.shape, y.dtype)


# revision 17
# speedup vs baseline: 1.1933x; 1.0168x over previous
"""AttentativeResidual Trainium2 kernel.

out[b,t,n,:] = x[b,t,n,:] + softmax_m(x[b,t,n,:] @ Wq @ Wk^T @ rs[b]^T) @ (rs[b] @ Wv)

Shapes: x [4,8,2048,128], residual_source [4,2048,128], W* [128,128], fp32.

Sharding: data-parallel over (b,t): core i handles b = i//2, t in
[(i%2)*4, (i%2)*4+4). Each core sees one batch b, so the per-batch
K/V-side work (rs^T, G = A @ rs^T, v = rs @ Wv) is computed once per core.

Math trick: fold A = Wq @ Wk^T (host-side, [128,128]) so the logits are
x @ A @ rs^T; on-device GT = A @ rsT ([c,m]) replaces both q and k
projections.

Device algorithm per core (flash-attention style, transposed logits):
  setup:  rsT (PE transposes, fp32r), GT = A@rsT (fp32r), v_aug =
          [rs@Wv | 1] (bf16 matmul, ones col)
  per (t, rb-half): per m-tile (16): aff^T[m-part, n] = GT_m^T @ xT
          (fp32r matmul, fp32 psum [128,1024]); exp -> bf16 ea tile,
          split across engines: ACT does true exp on 12/16 tiles; DVE
          and Pool each do 2/16 via a Schraudolph bit-trick (see below)
  AV (software-pipelined into the NEXT half's m-slots so PE never
  idles): per r-subtile: av[r,129] = sum_m ea[m][:,rsub]^T @ v_aug[m]
          (bf16, psum-accumulated; col 128 = softmax denominator)
          out[r,:] = av[:,:128] * (1/av[:,128]) + x[r,:]  (one fused
          DVE pass via the stock AFFINE_THEN_ADD custom op)

exp is computed without max-subtraction: logits ~ N(0, 128), |l| < ~75
with overwhelming probability, exp fits fp32/bf16 range. ea/v in bf16
because unnormalized exp(l) overflows fp16.

Fast exp (DVE/Pool tiles): exp(l) ~= bitcast_bf16(int16(l * 128*log2e
+ 127*128 + c)). The fp32->int16 output conversion truncates, so the
+0.5 is folded into c; the int16 bit pattern IS the bf16 exponent+
mantissa of 2^(l*log2e) with the mantissa linearly interpolated
(~+-4% sawtooth). End-to-end rel err measured 0.9e-2 vs the 2e-2 gate
(softmax weights are ratios, so the common-mode part cancels).

Schedule per (t, rb): the m-loop interleaves, per slot: 1 aff matmul
(1024 cols), the exp for that tile, and on odd slots one AV r-chunk of
the PREVIOUS half (16 accumulating matmuls); rb==1 slots also carry one
transpose of the next t's x tile. This keeps PE (the bottleneck at
~14.3us/half) fed while psA double-buffering paces the exp engines.
"""
import numpy as np

import concourse.bacc as bacc
import concourse.tile as tile
import concourse.mybir as mybir
from concourse.bass_utils import run_bass_kernel_spmd
from concourse.dve_ops import AFFINE_THEN_ADD
from concourse.masks import make_identity

F32 = mybir.dt.float32
F32R = mybir.dt.float32r
I16 = mybir.dt.int16
BF16 = mybir.dt.bfloat16
EXP = mybir.ActivationFunctionType.Exp
MULT = mybir.AluOpType.mult
ADD = mybir.AluOpType.add

B, T, N, C = 4, 8, 2048, 128
NCORES = 8
TPC = (B * T) // NCORES          # (b,t) pairs per core = 4
NT = N // 128                    # 16 row/key tiles

# Schraudolph fast-exp constants (bf16 bit pattern via int16 convert).
EXP_A = float(128.0 * np.log2(np.e))
EXP_B = float(127.0 * 128.0 + 0.5 - 6.0)
# m-slot -> exp engine: 'A' = ACT true exp, 'D' = DVE fast, 'P' = Pool fast.
EXP_ENG = ['A'] * NT
EXP_ENG[3] = EXP_ENG[6] = EXP_ENG[9] = EXP_ENG[12] = EXP_ENG[15] = 'D'


def _body(ctx, tc, xs, rs, at, wv, out):
    nc = tc.nc
    const = ctx.enter_context(tc.tile_pool(name="const", bufs=1))
    xpool = ctx.enter_context(tc.tile_pool(name="xp", bufs=2))
    xtp = ctx.enter_context(tc.tile_pool(name="xtp", bufs=2))
    eap = ctx.enter_context(tc.tile_pool(name="eap", bufs=34))
    outp = ctx.enter_context(tc.tile_pool(name="outp", bufs=3))
    recp = ctx.enter_context(tc.tile_pool(name="recp", bufs=8))
    # psA: 4 x 1-bank aff tiles -- the 4-deep rotation halves the
    # aff->exp->aff WAR chain latency vs 2 x [128,1024].
    psA = ctx.enter_context(tc.tile_pool(name="psA", bufs=4, space="PSUM"))
    psB = ctx.enter_context(tc.tile_pool(name="psB", bufs=2, space="PSUM"))
    psC = ctx.enter_context(tc.tile_pool(name="psC", bufs=2, space="PSUM"))

    xr = xs[:, :, :].rearrange("t (i p) c -> t p i c", p=128)
    outr = out[:, :, :].rearrange("t (i p) c -> t p i c", p=128)

    ident = const.tile([128, 128], F32, tag="ident")
    make_identity(nc, ident)
    # ACT HWDGE queue order: rs chunk 0 (gates the first PE transpose),
    # then tiny at/wv (their DVE copies head the DVE queue), then the
    # rest of rs; x goes on the SP queue in parallel.
    at_sb = const.tile([128, 128], F32, tag="at")
    wv_sb = const.tile([128, 128], F32, tag="wv")
    rs_sb = const.tile([128, NT, 128], F32, tag="rs")
    rsr = rs[:, :].rearrange("(i p) c -> p i c", p=128)
    nc.scalar.dma_start(out=at_sb, in_=at[:, :])
    nc.scalar.dma_start(out=wv_sb, in_=wv[:, :])
    for j in range(4):
        nc.scalar.dma_start(out=rs_sb[:, 4 * j:4 * j + 4, :],
                            in_=rsr[:, 4 * j:4 * j + 4, :])
    at_r = const.tile([128, 128], F32R, tag="atr")
    wv_b = const.tile([128, 128], BF16, tag="wvb")
    nc.vector.tensor_copy(out=at_r, in_=at_sb)
    nc.gpsimd.tensor_copy(out=wv_b, in_=wv_sb)

    xss = {}
    xts = {}

    def issue_x_dma(t, split=1):
        xss[t] = xpool.tile([128, NT, 128], F32, tag="x", name="x_sb")
        step = NT // split
        for j in range(split):
            nc.sync.dma_start(out=xss[t][:, step * j:step * (j + 1), :],
                              in_=xr[t][:, step * j:step * (j + 1), :])

    tgroups = {}

    def emit_transpose(t, i, pool, tag, copy_eng):
        # Groups of 4 transposes share one psum tile; a single 512-col
        # copy drains the group (fewer, wider DVE instructions).
        g = tgroups.get((pool is psA, tag))
        if g is None or g[2] == 4:
            tp = pool.tile([128, 512], F32, tag=tag, name="tp")
            g = [tp, i, 0]
        nc.tensor.transpose(g[0][:, 128 * g[2]:128 * (g[2] + 1)],
                            xss[t][:, i, :], ident)
        g[2] += 1
        tgroups[(pool is psA, tag)] = g
        if g[2] == 4:
            i0 = g[1]
            copy_eng.tensor_copy(out=xts[t][:, i0:i0 + 4, :], in_=g[0])

    issue_x_dma(0, split=4)

    # --- per-batch setup: rsT (fp32r), GT (fp32r); x0 transposes
    # interleave on the psA pool so both PSUM chains run in parallel.
    # GT(j) is emitted as soon as its 4 rsT tiles land; GT psum->SBUF
    # copies go to ACT (idle during setup) to keep DVE off the chain.
    rsT_sb = const.tile([128, NT, 128], F32R, tag="rsT")
    gt_sb = const.tile([128, NT, 128], F32R, tag="gt")
    rsT_bf = const.tile([128, NT, 128], BF16, tag="rsTb")
    xts[0] = xtp.tile([128, NT, 128], F32R, tag="xt", name="xt_sb")
    rtp = None
    for i in range(NT):
        if i % 4 == 0:
            rtp = psC.tile([128, 512], F32, tag="misc", name="rtp")
        nc.tensor.transpose(rtp[:, 128 * (i % 4):128 * (i % 4 + 1)],
                            rs_sb[:, i, :], ident)
        if i < 8:
            emit_transpose(0, i, psA, "aff", nc.vector)
        if i % 4 == 3:
            j = i // 4
            nc.vector.tensor_copy(out=rsT_sb[:, 4 * j:4 * j + 4, :], in_=rtp)
            nc.gpsimd.tensor_copy(out=rsT_bf[:, 4 * j:4 * j + 4, :],
                                  in_=rsT_sb[:, 4 * j:4 * j + 4, :])
            gp = psC.tile([128, 512], F32, tag="misc")
            nc.tensor.matmul(gp, at_r, rsT_sb[:, 4 * j:4 * j + 4, :],
                             start=True, stop=True)
            nc.vector.tensor_copy(out=gt_sb[:, 4 * j:4 * j + 4, :], in_=gp)

    # v = rs @ Wv runs in bf16 during the ACT-bound first half (PE has
    # slack there); v_aug holds [v | 1] with the softmax-denominator col.
    # The rsT f32r->bf16 casts are SBUF-only, so the otherwise-idle Pool
    # engine does them (GPSIMD cannot touch PSUM).
    v_aug = const.tile([128, NT, 129], BF16, tag="vaug")

    vgroup = [None]

    def emit_v(i0, i1):
        for i in range(i0, i1):
            if i % 4 == 0:
                vgroup[0] = psC.tile([128, 512], F32, tag="misc", name="vp")
            vp = vgroup[0]
            nc.tensor.matmul(vp[:, 128 * (i % 4):128 * (i % 4 + 1)],
                             rsT_bf[:, i, :], wv_b, start=True, stop=True)
            if i % 4 == 3:
                nc.vector.tensor_copy(out=v_aug[:, i - 3:i + 1, 0:128], in_=vp)
        if i1 == NT:
            nc.vector.memset(v_aug[:, :, 128:129], 1.0)

    # One AV r-chunk of a previous half: 16 accumulating bf16 matmuls,
    # then out = av * (1/denominator) + x in one fused DVE op. Emitted in
    # two pieces (two consecutive m-slots) so PE work stays evenly paced.
    av_live = {}

    def emit_av_piece(ph, k, piece):
        t_, rb_, eas, x_sb_, out_sb = ph
        if piece == 0:
            av_live[k] = psB.tile([128, 129], F32, tag="av", name="av")
        av = av_live[k]
        m0 = piece * (NT // 2)
        for m in range(m0, m0 + NT // 2):
            nc.tensor.matmul(av, eas[m][:, 128 * k:128 * (k + 1)],
                             v_aug[:, m, :],
                             start=(m == 0), stop=(m == NT - 1))
        if piece == 1:
            rec = recp.tile([128, 1], F32, tag="rec")
            nc.vector.reciprocal(out=rec, in_=av[:, 128:129])
            nc.vector._custom_dve(AFFINE_THEN_ADD, out=out_sb[:, k, :],
                                  in0=av[:, 0:128],
                                  in1=x_sb_[:, 8 * rb_ + k, :],
                                  s0=rec, s1=0.0)
            if k == 3 or k == 7 or (k == 5 and t_ == TPC - 1 and rb_ == 1):
                k0 = 4 if k == 5 else k - 3
                n = 2 if k >= 5 and t_ == TPC - 1 and rb_ == 1 else 4
                k0 = k - n + 1
                nc.sync.dma_start(
                    out=outr[t_][:, 8 * rb_ + k0:8 * rb_ + k0 + n, :],
                    in_=out_sb[:, k0:k0 + n, :])

    def emit_exp(eng, ea_half, ap):
        if eng == 'A':
            nc.scalar.activation(out=ea_half, in_=ap, func=EXP)
        else:
            veng = nc.vector if eng == 'D' else nc.gpsimd
            veng.tensor_scalar(out=ea_half.bitcast(I16), in0=ap,
                               scalar1=EXP_A, scalar2=EXP_B,
                               op0=MULT, op1=ADD)

    prev = None
    for t in range(TPC):
        if t + 1 < TPC:
            issue_x_dma(t + 1)
        for rb in range(2):
            if rb == 1 and t + 1 < TPC:
                xts[t + 1] = xtp.tile([128, NT, 128], F32R, tag="xt",
                                      name="xt_sb")
            eas_h = []
            first_half = prev is None
            last_half = (t == TPC - 1 and rb == 1)
            tail_avs = {}
            for m in range(NT):
                ea = eap.tile([128, 1024], BF16, tag="ea")
                eng = EXP_ENG[m]
                for h in range(2):
                    ap = psA.tile([128, 512], F32, tag="aff", name="ap")
                    nc.tensor.matmul(
                        ap, gt_sb[:, m, :],
                        xts[t][:, 8 * rb + 4 * h:8 * rb + 4 * (h + 1), :],
                        start=True, stop=True)
                    emit_exp(eng, ea[:, 512 * h:512 * (h + 1)], ap)
                eas_h.append(ea)
                if last_half and m >= 1:
                    # The final half's first two AV chunks accumulate
                    # inline (psC is free: no next-t transposes), shrinking
                    # the drain tail after the loop.
                    for k in range(2):
                        if m == 1:
                            tail_avs[k] = psC.tile([128, 129], F32,
                                                   tag="misc", name="avt")
                        nc.tensor.matmul(tail_avs[k],
                                         eas_h[m - 1][:, 128 * k:128 * (k + 1)],
                                         v_aug[:, m - 1, :],
                                         start=(m == 1), stop=False)
                if prev is not None:
                    emit_av_piece(prev, m // 2, m % 2)
                else:
                    if m % 2 == 1:
                        emit_v(m - 1, m + 1)
                    if m % 2 == 0:
                        emit_transpose(0, 8 + m // 2, psC, "misc", nc.vector)
                if rb == 1 and t + 1 < TPC:
                    emit_transpose(t + 1, m, psC, "misc", nc.vector)
            out_sb = outp.tile([128, 8, 128], F32, tag="o")
            prev = (t, rb, eas_h, xss[t], out_sb)
    t_, rb_, eas, x_sb_, out_sb = prev
    for k in range(2):
        nc.tensor.matmul(tail_avs[k], eas[NT - 1][:, 128 * k:128 * (k + 1)],
                         v_aug[:, NT - 1, :], start=False, stop=True)
        rec = recp.tile([128, 1], F32, tag="rec")
        nc.vector.reciprocal(out=rec, in_=tail_avs[k][:, 128:129])
        nc.vector._custom_dve(AFFINE_THEN_ADD, out=out_sb[:, k, :],
                              in0=tail_avs[k][:, 0:128],
                              in1=x_sb_[:, 8 * rb_ + k, :],
                              s0=rec, s1=0.0)
    for k in range(2, 8):
        emit_av_piece(prev, k, 0)
        emit_av_piece(prev, k, 1)


def _run_on_cores(nc, in_maps):
    """Run the bass module on len(in_maps) NeuronCores as independent
    single-device programs dispatched concurrently.

    run_bass_kernel_spmd's multi-core path lowers to one shard_map program
    spanning 8 devices, which deadlocks through the axon PJRT tunnel in this
    environment. Independent per-device jits of the same bass_exec body work
    (and still run concurrently on all 8 cores), so we dispatch those.
    """
    import jax
    from concourse import bass2jax

    bass2jax.install_neuronx_cc_hook()
    devices = jax.devices()[:len(in_maps)]
    assert len(devices) == len(in_maps)

    partition_name = (nc.partition_id_tensor.name
                      if nc.partition_id_tensor else None)
    dbg_name = nc.dbg_addr.name if nc.dbg_addr is not None else None
    in_names, out_names, out_avals, zero_outs = [], [], [], []
    for alloc in nc.m.functions[0].allocations:
        if not isinstance(alloc, mybir.MemoryLocationSet):
            continue
        name = alloc.memorylocations[0].name
        if alloc.kind == "ExternalInput":
            if name != partition_name:
                in_names.append(name)
        elif alloc.kind == "ExternalOutput":
            shape = tuple(alloc.tensor_shape)
            dtype = mybir.dt.np(alloc.dtype)
            out_names.append(name)
            out_avals.append(jax.core.ShapedArray(shape, dtype))
            zero_outs.append(np.zeros(shape, dtype))

    n_params = len(in_names)
    in_names_all = tuple(in_names + out_names + (
        [partition_name] if partition_name else []))
    donate = tuple(range(n_params, n_params + len(out_names)))

    def _bass_body(*args):
        operands = list(args)
        if partition_name is not None:
            operands.append(bass2jax.partition_id_tensor())
        outs = bass2jax._bass_exec_p.bind(
            *operands,
            out_avals=tuple(out_avals),
            in_names=in_names_all,
            out_names=tuple(out_names),
            lowering_input_output_aliases=(),
            sim_require_finite=True,
            sim_require_nnan=True,
            nc=nc,
        )
        return tuple(outs)

    jf = jax.jit(_bass_body, donate_argnums=donate, keep_unused=True)
    futs = []
    for c, im in enumerate(in_maps):
        im = dict(im)
        if dbg_name is not None:
            im[dbg_name] = np.zeros((1, 2), np.uint32)
        args = [jax.device_put(np.asarray(im[n]), devices[c])
                for n in in_names]
        args += [jax.device_put(z, devices[c]) for z in zero_outs]
        futs.append(jf(*args))
    return [{n: np.asarray(outs[i]) for i, n in enumerate(out_names)}
            for outs in futs]


_NC_CACHE = None


def _get_nc():
    global _NC_CACHE
    if _NC_CACHE is None:
        nc = bacc.Bacc("TRN2", target_bir_lowering=False)
        xs = nc.dram_tensor("xs", [TPC, N, C], F32, kind="ExternalInput")
        rs = nc.dram_tensor("rs", [N, C], F32, kind="ExternalInput")
        at = nc.dram_tensor("at", [C, C], F32, kind="ExternalInput")
        wv = nc.dram_tensor("wv", [C, C], F32, kind="ExternalInput")
        out = nc.dram_tensor("out", [TPC, N, C], F32, kind="ExternalOutput")
        from contextlib import ExitStack
        with tile.TileContext(nc) as tc, ExitStack() as ctx:
            _body(ctx, tc, xs, rs, at, wv, out)
        nc.finalize()
        _NC_CACHE = nc
    return _NC_CACHE


def kernel(x, residual_source, Wq, Wk, Wv):
    x = np.asarray(x, dtype=np.float32)
    residual_source = np.asarray(residual_source, dtype=np.float32)
    Wq = np.asarray(Wq, dtype=np.float32)
    Wk = np.asarray(Wk, dtype=np.float32)
    Wv = np.asarray(Wv, dtype=np.float32)

    at = np.ascontiguousarray(Wk @ Wq.T)  # A^T where A = Wq @ Wk^T
    nc = _get_nc()

    in_maps = []
    for core in range(NCORES):
        b, toff = core // 2, (core % 2) * TPC
        in_maps.append({
            "xs": np.ascontiguousarray(x[b, toff:toff + TPC]),
            "rs": np.ascontiguousarray(residual_source[b]),
            "at": at,
            "wv": np.ascontiguousarray(Wv),
        })
    results = _run_on_cores(nc, in_maps)

    out = np.empty((B, T, N, C), np.float32)
    for core in range(NCORES):
        b, toff = core // 2, (core % 2) * TPC
        out[b, toff:toff + TPC] = results[core]["out"]
    return out


if __name__ == "__main__":
    rng = np.random.default_rng(0)
    x = rng.standard_normal((B, T, N, C)).astype(np.float32)
    rs = rng.standard_normal((B, N, C)).astype(np.float32)
    s = 1.0 / np.sqrt(C)
    Wq = (rng.standard_normal((C, C)) * s).astype(np.float32)
    Wk = (rng.standard_normal((C, C)) * s).astype(np.float32)
    Wv = (rng.standard_normal((C, C)) * s).astype(np.float32)
    y = kernel(x, rs, Wq, Wk, Wv)
    print("out", y.shape, y.dtype)


# revision 24
# speedup vs baseline: 1.2174x; 1.0202x over previous
"""AttentativeResidual Trainium2 kernel.

out[b,t,n,:] = x[b,t,n,:] + softmax_m(x[b,t,n,:] @ Wq @ Wk^T @ rs[b]^T) @ (rs[b] @ Wv)

Shapes: x [4,8,2048,128], residual_source [4,2048,128], W* [128,128], fp32.

Sharding: data-parallel over (b,t): core i handles b = i//2, t in
[(i%2)*4, (i%2)*4+4). Each core sees one batch b, so the per-batch
K/V-side work (rs^T, G = A @ rs^T, v = rs @ Wv) is computed once per core.

Math trick: fold A = Wq @ Wk^T (host-side, [128,128]) so the logits are
x @ A @ rs^T; on-device GT = A @ rsT ([c,m]) replaces both q and k
projections.

Device algorithm per core (flash-attention style, transposed logits):
  setup:  rsT via PE transposes (4 per psum tile, one wide DVE copy
          each + a Pool bf16 cast for the v matmul), GT = A@rsT (fp32r),
          v_aug = [rs@Wv | 1] bf16 (ones col = softmax denominator)
  per (t, rb-half), m-slot 0..15: one 512-col fp32r aff matmul pair
          aff^T[m-part, n] = GT_m^T @ xT into a 4-deep rotation of
          1-bank psum tiles (halves the aff->exp->aff WAR chain vs
          2x[128,1024]); exp -> bf16 ea tile, split ACT 11 / DVE 5
          (GPSIMD cannot touch PSUM, so Pool gets only SBUF work)
  AV, software-pipelined into the NEXT half's m-slots so PE never
  idles: per r-chunk: av[r,129] = sum_m ea[m][:,r]^T @ v_aug[m] (bf16,
          psum-accumulated, split into two 8-matmul pieces per slot);
          out[r,:] = av[:,:128] * (1/av[:,128]) + x[r,:] in ONE fused
          DVE pass (stock AFFINE_THEN_ADD custom op); the final half
          accumulates its first two chunks inline in the then-idle psC
          pool to shrink the drain tail.

exp is computed without max-subtraction: logits ~ N(0, 128), |l| < ~75
with overwhelming probability, exp fits fp32/bf16 range. ea/v in bf16
because unnormalized exp(l) overflows fp16.

Fast exp (DVE tiles): exp(l) ~= bitcast_bf16(int16(l * 128*log2e +
127*128 + c)) via a single dual-op tensor_scalar whose fp32->int16
output conversion truncates (the +0.5 is folded into c). The int16 bit
pattern IS the bf16 exponent+mantissa of 2^(l*log2e) with the mantissa
linearly interpolated (~+-4% sawtooth). Softmax weights are ratios, so
the common-mode part cancels; end-to-end rel err ~0.9e-2 vs the 2e-2
gate (exact-exp pipeline: ~0.3e-2).

Schedule per (t, rb): each m-slot issues aff+exp for both 512-col
halves, one 8-matmul AV piece of the previous half, and (rb==1) one
transpose of the next t's x tile (grouped 4 per psum tile, one 512-col
DVE copy). PE is the bottleneck engine (~89% busy); ACT runs the true
exps (~79%), DVE runs fast-exp + epilogue + psum drains (~64%).
"""
import numpy as np

import concourse.bacc as bacc
import concourse.tile as tile
import concourse.mybir as mybir
from concourse.bass_utils import run_bass_kernel_spmd
from concourse.dve_ops import AFFINE_THEN_ADD
from concourse.masks import make_identity

F32 = mybir.dt.float32
F32R = mybir.dt.float32r
I16 = mybir.dt.int16
BF16 = mybir.dt.bfloat16
EXP = mybir.ActivationFunctionType.Exp
MULT = mybir.AluOpType.mult
ADD = mybir.AluOpType.add

B, T, N, C = 4, 8, 2048, 128
NCORES = 8
TPC = (B * T) // NCORES          # (b,t) pairs per core = 4
NT = N // 128                    # 16 row/key tiles

# Schraudolph fast-exp constants (bf16 bit pattern via int16 convert).
EXP_A = float(128.0 * np.log2(np.e))
EXP_B = float(127.0 * 128.0 + 0.5 - 6.0)
# m-slot -> exp engine: 'A' = ACT true exp, 'D' = DVE fast-exp.
EXP_ENG = ['A'] * NT
EXP_ENG[1] = EXP_ENG[4] = EXP_ENG[7] = EXP_ENG[10] = EXP_ENG[13] = 'D'


def _body(ctx, tc, xs, rs, at, wv, out):
    nc = tc.nc
    const = ctx.enter_context(tc.tile_pool(name="const", bufs=1))
    xpool = ctx.enter_context(tc.tile_pool(name="xp", bufs=2))
    xtp = ctx.enter_context(tc.tile_pool(name="xtp", bufs=2))
    eap = ctx.enter_context(tc.tile_pool(name="eap", bufs=36))
    outp = ctx.enter_context(tc.tile_pool(name="outp", bufs=3))
    recp = ctx.enter_context(tc.tile_pool(name="recp", bufs=8))
    # psA: 4 x 1-bank aff tiles -- the 4-deep rotation halves the
    # aff->exp->aff WAR chain latency vs 2 x [128,1024].
    psA = ctx.enter_context(tc.tile_pool(name="psA", bufs=4, space="PSUM"))
    psB = ctx.enter_context(tc.tile_pool(name="psB", bufs=2, space="PSUM"))
    psC = ctx.enter_context(tc.tile_pool(name="psC", bufs=2, space="PSUM"))

    xr = xs[:, :, :].rearrange("t (i p) c -> t p i c", p=128)
    outr = out[:, :, :].rearrange("t (i p) c -> t p i c", p=128)

    ident = const.tile([128, 128], F32, tag="ident")
    make_identity(nc, ident)
    # PE p-state warmup: dummy transposes of ident while the first DMAs
    # are in flight, so the 3us ramp to full clock burns idle time, not
    # real work. Outputs are never read.
    for _ in range(14):
        wtp = psA.tile([128, 512], F32, tag="aff", name="wtp")
        nc.tensor.transpose(wtp[:, 0:128], ident, ident)
    # ACT HWDGE queue: tiny at/wv first (their copies head the DVE/Pool
    # queues), then rs chunk 0 (gates the first rs transpose); x and the
    # remaining rs chunks ride the SP queue in parallel.
    at_sb = const.tile([128, 128], F32, tag="at")
    wv_sb = const.tile([128, 128], F32, tag="wv")
    rs_sb = const.tile([128, NT, 128], F32, tag="rs")
    rsr = rs[:, :].rearrange("(i p) c -> p i c", p=128)
    nc.scalar.dma_start(out=at_sb, in_=at[:, :])
    nc.scalar.dma_start(out=wv_sb, in_=wv[:, :])
    nc.scalar.dma_start(out=rs_sb[:, 0:4, :], in_=rsr[:, 0:4, :])
    at_r = const.tile([128, 128], F32R, tag="atr")
    wv_b = const.tile([128, 128], BF16, tag="wvb")
    nc.vector.tensor_copy(out=at_r, in_=at_sb)
    nc.gpsimd.tensor_copy(out=wv_b, in_=wv_sb)

    xss = {}
    xts = {}

    def issue_x_dma(t, split=1):
        xss[t] = xpool.tile([128, NT, 128], F32, tag="x", name="x_sb")
        step = NT // split
        for j in range(split):
            nc.sync.dma_start(out=xss[t][:, step * j:step * (j + 1), :],
                              in_=xr[t][:, step * j:step * (j + 1), :])

    tgroups = {}

    def emit_transpose(t, i, pool, tag, copy_eng):
        # Groups of 4 transposes share one psum tile; a single 512-col
        # copy drains the group (fewer, wider DVE instructions).
        g = tgroups.get((pool is psA, tag))
        if g is None or g[2] == 4:
            tp = pool.tile([128, 512], F32, tag=tag, name="tp")
            g = [tp, i, 0]
        nc.tensor.transpose(g[0][:, 128 * g[2]:128 * (g[2] + 1)],
                            xss[t][:, i, :], ident)
        g[2] += 1
        tgroups[(pool is psA, tag)] = g
        if g[2] == 4:
            i0 = g[1]
            copy_eng.tensor_copy(out=xts[t][:, i0:i0 + 4, :], in_=g[0])

    issue_x_dma(0, split=4)
    # rs chunks 1-3 ride the SP queue behind x0 -- lands sooner than
    # serializing behind at/wv/rs0 on the ACT HWDGE queue.
    for j in range(1, 4):
        nc.sync.dma_start(out=rs_sb[:, 4 * j:4 * j + 4, :],
                          in_=rsr[:, 4 * j:4 * j + 4, :])

    # --- per-batch setup: rsT (fp32r), GT (fp32r); x0 transposes
    # interleave on the psA pool so both PSUM chains run in parallel.
    # GT(j) is emitted as soon as its 4 rsT tiles land.
    rsT_sb = const.tile([128, NT, 128], F32R, tag="rsT")
    gt_sb = const.tile([128, NT, 128], F32R, tag="gt")
    rsT_bf = const.tile([128, NT, 128], BF16, tag="rsTb")
    xts[0] = xtp.tile([128, NT, 128], F32R, tag="xt", name="xt_sb")
    # x0 group 0 leads (x chunk 0 lands ~1.3us, well before rs) so PE
    # starts early and the p-state ramp begins; the second x0 group
    # trails GT group 0 so its DVE copy queues after the gt copy (gt
    # gates the first aff matmuls).
    for u in range(4):
        emit_transpose(0, u, psA, "aff", nc.vector)
    rtp = None
    for i in range(NT):
        if i % 4 == 0:
            rtp = psC.tile([128, 512], F32, tag="misc", name="rtp")
        nc.tensor.transpose(rtp[:, 128 * (i % 4):128 * (i % 4 + 1)],
                            rs_sb[:, i, :], ident)
        if i % 4 == 3:
            j = i // 4
            nc.vector.tensor_copy(out=rsT_sb[:, 4 * j:4 * j + 4, :], in_=rtp)
            nc.gpsimd.tensor_copy(out=rsT_bf[:, 4 * j:4 * j + 4, :],
                                  in_=rsT_sb[:, 4 * j:4 * j + 4, :])
            gp = psC.tile([128, 512], F32, tag="misc")
            nc.tensor.matmul(gp, at_r, rsT_sb[:, 4 * j:4 * j + 4, :],
                             start=True, stop=True)
            nc.vector.tensor_copy(out=gt_sb[:, 4 * j:4 * j + 4, :], in_=gp)
            if i == 3:
                for u in range(4, 8):
                    emit_transpose(0, u, psA, "aff", nc.vector)

    # v = rs @ Wv runs in bf16 during the ACT-bound first half (PE has
    # slack there); v_aug holds [v | 1] with the softmax-denominator col.
    # The rsT f32r->bf16 casts are SBUF-only, so the otherwise-idle Pool
    # engine does them (GPSIMD cannot touch PSUM).
    v_aug = const.tile([128, NT, 129], BF16, tag="vaug")

    vgroup = [None]

    def emit_v(i0, i1):
        for i in range(i0, i1):
            if i % 4 == 0:
                vgroup[0] = psC.tile([128, 512], F32, tag="misc", name="vp")
            vp = vgroup[0]
            nc.tensor.matmul(vp[:, 128 * (i % 4):128 * (i % 4 + 1)],
                             rsT_bf[:, i, :], wv_b, start=True, stop=True)
            if i % 4 == 3:
                nc.vector.tensor_copy(out=v_aug[:, i - 3:i + 1, 0:128], in_=vp)
        if i1 == NT:
            nc.vector.memset(v_aug[:, :, 128:129], 1.0)

    # One AV r-chunk of a previous half: 16 accumulating bf16 matmuls,
    # then out = av * (1/denominator) + x in one fused DVE op. Emitted in
    # two pieces (two consecutive m-slots) so PE work stays evenly paced.
    av_live = {}

    def emit_av_piece(ph, k, piece):
        t_, rb_, eas, x_sb_, out_sb = ph
        if piece == 0:
            av_live[k] = psB.tile([128, 129], F32, tag="av", name="av")
        av = av_live[k]
        m0 = piece * (NT // 2)
        for m in range(m0, m0 + NT // 2):
            nc.tensor.matmul(av, eas[m][:, 128 * k:128 * (k + 1)],
                             v_aug[:, m, :],
                             start=(m == 0), stop=(m == NT - 1))
        if piece == 1:
            rec = recp.tile([128, 1], F32, tag="rec")
            nc.vector.reciprocal(out=rec, in_=av[:, 128:129])
            nc.vector._custom_dve(AFFINE_THEN_ADD, out=out_sb[:, k, :],
                                  in0=av[:, 0:128],
                                  in1=x_sb_[:, 8 * rb_ + k, :],
                                  s0=rec, s1=0.0)
            last = t_ == TPC - 1 and rb_ == 1
            if k == 3 or k == 7 or (k == 5 and last):
                n = 2 if k >= 5 and last else 4
                k0 = k - n + 1
                nc.sync.dma_start(
                    out=outr[t_][:, 8 * rb_ + k0:8 * rb_ + k0 + n, :],
                    in_=out_sb[:, k0:k0 + n, :])

    def emit_exp(eng, ea_half, ap):
        if eng == 'A':
            nc.scalar.activation(out=ea_half, in_=ap, func=EXP)
        else:
            veng = nc.vector if eng == 'D' else nc.gpsimd
            veng.tensor_scalar(out=ea_half.bitcast(I16), in0=ap,
                               scalar1=EXP_A, scalar2=EXP_B,
                               op0=MULT, op1=ADD)

    prev = None
    for t in range(TPC):
        if t + 1 < TPC:
            issue_x_dma(t + 1)
        for rb in range(2):
            if rb == 1 and t + 1 < TPC:
                xts[t + 1] = xtp.tile([128, NT, 128], F32R, tag="xt",
                                      name="xt_sb")
            eas_h = []
            first_half = prev is None
            last_half = (t == TPC - 1 and rb == 1)
            tail_avs = {}
            for m in range(NT):
                ea = eap.tile([128, 1024], BF16, tag="ea")
                eng = EXP_ENG[m]
                for h in range(2):
                    ap = psA.tile([128, 512], F32, tag="aff", name="ap")
                    nc.tensor.matmul(
                        ap, gt_sb[:, m, :],
                        xts[t][:, 8 * rb + 4 * h:8 * rb + 4 * (h + 1), :],
                        start=True, stop=True)
                    emit_exp(eng, ea[:, 512 * h:512 * (h + 1)], ap)
                eas_h.append(ea)
                if last_half and m >= 1:
                    # The final half's first two AV chunks accumulate
                    # inline (psC is free: no next-t transposes), shrinking
                    # the drain tail after the loop.
                    for k in range(2):
                        if m == 1:
                            tail_avs[k] = psC.tile([128, 129], F32,
                                                   tag="misc", name="avt")
                        nc.tensor.matmul(tail_avs[k],
                                         eas_h[m - 1][:, 128 * k:128 * (k + 1)],
                                         v_aug[:, m - 1, :],
                                         start=(m == 1), stop=False)
                if prev is not None:
                    emit_av_piece(prev, m // 2, m % 2)
                else:
                    if m % 2 == 1:
                        emit_v(m - 1, m + 1)
                    if m % 2 == 0:
                        emit_transpose(0, 8 + m // 2, psC, "misc", nc.vector)
                if rb == 1 and t + 1 < TPC:
                    emit_transpose(t + 1, m, psC, "misc", nc.vector)
            out_sb = outp.tile([128, 8, 128], F32, tag="o")
            prev = (t, rb, eas_h, xss[t], out_sb)
    t_, rb_, eas, x_sb_, out_sb = prev
    for k in range(2):
        nc.tensor.matmul(tail_avs[k], eas[NT - 1][:, 128 * k:128 * (k + 1)],
                         v_aug[:, NT - 1, :], start=False, stop=True)
        rec = recp.tile([128, 1], F32, tag="rec")
        nc.vector.reciprocal(out=rec, in_=tail_avs[k][:, 128:129])
        nc.vector._custom_dve(AFFINE_THEN_ADD, out=out_sb[:, k, :],
                              in0=tail_avs[k][:, 0:128],
                              in1=x_sb_[:, 8 * rb_ + k, :],
                              s0=rec, s1=0.0)
    for k in range(2, 8):
        emit_av_piece(prev, k, 0)
        emit_av_piece(prev, k, 1)


def _run_on_cores(nc, in_maps):
    """Run the bass module on len(in_maps) NeuronCores as independent
    single-device programs dispatched concurrently.

    run_bass_kernel_spmd's multi-core path lowers to one shard_map program
    spanning 8 devices, which deadlocks through the axon PJRT tunnel in this
    environment. Independent per-device jits of the same bass_exec body work
    (and still run concurrently on all 8 cores), so we dispatch those.
    """
    import jax
    from concourse import bass2jax

    bass2jax.install_neuronx_cc_hook()
    devices = jax.devices()[:len(in_maps)]
    assert len(devices) == len(in_maps)

    partition_name = (nc.partition_id_tensor.name
                      if nc.partition_id_tensor else None)
    dbg_name = nc.dbg_addr.name if nc.dbg_addr is not None else None
    in_names, out_names, out_avals, zero_outs = [], [], [], []
    for alloc in nc.m.functions[0].allocations:
        if not isinstance(alloc, mybir.MemoryLocationSet):
            continue
        name = alloc.memorylocations[0].name
        if alloc.kind == "ExternalInput":
            if name != partition_name:
                in_names.append(name)
        elif alloc.kind == "ExternalOutput":
            shape = tuple(alloc.tensor_shape)
            dtype = mybir.dt.np(alloc.dtype)
            out_names.append(name)
            out_avals.append(jax.core.ShapedArray(shape, dtype))
            zero_outs.append(np.zeros(shape, dtype))

    n_params = len(in_names)
    in_names_all = tuple(in_names + out_names + (
        [partition_name] if partition_name else []))
    donate = tuple(range(n_params, n_params + len(out_names)))

    def _bass_body(*args):
        operands = list(args)
        if partition_name is not None:
            operands.append(bass2jax.partition_id_tensor())
        outs = bass2jax._bass_exec_p.bind(
            *operands,
            out_avals=tuple(out_avals),
            in_names=in_names_all,
            out_names=tuple(out_names),
            lowering_input_output_aliases=(),
            sim_require_finite=True,
            sim_require_nnan=True,
            nc=nc,
        )
        return tuple(outs)

    jf = jax.jit(_bass_body, donate_argnums=donate, keep_unused=True)
    futs = []
    for c, im in enumerate(in_maps):
        im = dict(im)
        if dbg_name is not None:
            im[dbg_name] = np.zeros((1, 2), np.uint32)
        args = [jax.device_put(np.asarray(im[n]), devices[c])
                for n in in_names]
        args += [jax.device_put(z, devices[c]) for z in zero_outs]
        futs.append(jf(*args))
    return [{n: np.asarray(outs[i]) for i, n in enumerate(out_names)}
            for outs in futs]


_NC_CACHE = None


def _get_nc():
    global _NC_CACHE
    if _NC_CACHE is None:
        nc = bacc.Bacc("TRN2", target_bir_lowering=False)
        xs = nc.dram_tensor("xs", [TPC, N, C], F32, kind="ExternalInput")
        rs = nc.dram_tensor("rs", [N, C], F32, kind="ExternalInput")
        at = nc.dram_tensor("at", [C, C], F32, kind="ExternalInput")
        wv = nc.dram_tensor("wv", [C, C], F32, kind="ExternalInput")
        out = nc.dram_tensor("out", [TPC, N, C], F32, kind="ExternalOutput")
        from contextlib import ExitStack
        with tile.TileContext(nc) as tc, ExitStack() as ctx:
            _body(ctx, tc, xs, rs, at, wv, out)
        nc.finalize()
        _NC_CACHE = nc
    return _NC_CACHE


def kernel(x, residual_source, Wq, Wk, Wv):
    x = np.asarray(x, dtype=np.float32)
    residual_source = np.asarray(residual_source, dtype=np.float32)
    Wq = np.asarray(Wq, dtype=np.float32)
    Wk = np.asarray(Wk, dtype=np.float32)
    Wv = np.asarray(Wv, dtype=np.float32)

    at = np.ascontiguousarray(Wk @ Wq.T)  # A^T where A = Wq @ Wk^T
    nc = _get_nc()

    in_maps = []
    for core in range(NCORES):
        b, toff = core // 2, (core % 2) * TPC
        in_maps.append({
            "xs": np.ascontiguousarray(x[b, toff:toff + TPC]),
            "rs": np.ascontiguousarray(residual_source[b]),
            "at": at,
            "wv": np.ascontiguousarray(Wv),
        })
    results = _run_on_cores(nc, in_maps)

    out = np.empty((B, T, N, C), np.float32)
    for core in range(NCORES):
        b, toff = core // 2, (core % 2) * TPC
        out[b, toff:toff + TPC] = results[core]["out"]
    return out


if __name__ == "__main__":
    rng = np.random.default_rng(0)
    x = rng.standard_normal((B, T, N, C)).astype(np.float32)
    rs = rng.standard_normal((B, N, C)).astype(np.float32)
    s = 1.0 / np.sqrt(C)
    Wq = (rng.standard_normal((C, C)) * s).astype(np.float32)
    Wk = (rng.standard_normal((C, C)) * s).astype(np.float32)
    Wv = (rng.standard_normal((C, C)) * s).astype(np.float32)
    y = kernel(x, rs, Wq, Wk, Wv)
    print("out", y.shape, y.dtype)

